# revision 1
# baseline (speedup 1.0000x reference)
"""Multi-headed attention with additive positional bias on 8 Trainium2 cores.

Sharding: data-parallel over batch (B=2) x tensor-parallel over head groups
(4 heads per core).  core = b*4 + hg handles batch b, heads [4*hg, 4*hg+4).

Per-core kernel (all matmuls in fp32r = full-rate tf32-like):
  phase 1: load weight shards (pre-transposed on host) into SBUF
  phase 2: projections
      qT/kT in head-transposed layout [qdim(128part) x 2 x S]
      v in natural layout, packed as v_aug [sk(128part) x 16 x (4 heads x 65)]
      with a ones-column per head (softmax denominator trick, produced by an
      extra zero weight column + bias=1 in the V projection)
  phase 3 (software-pipelined across units u=(sq-tile, head)):
      scoresT = kT_h^T q_h (k=64 matmul) -> += posr^T via DVE from PSUM ->
      exp on ACT in-place; the ctx matmul + normalization for unit u are
      emitted one unit later so the PE can run unit u+1's score matmuls
      while ACT computes exp(u).
      ctxT_aug = v_aug_h^T p (accumulate over sk); normalize by the ones-row
      denominator (reciprocal + gpsimd partition broadcast).
  phase 4: out = ctxT^T @ WoT shard per sq-tile as soon as its 4 heads
      finish, DMA to DRAM.

Host: fold 1/sqrt(dk) into Wq/bq, transpose inputs, sum the 4 per-batch
partial outputs + bo.  mask is all-ones by construction and ignored.
"""
import contextlib

import numpy as np

import concourse.bacc as bacc
import concourse.mybir as mybir
import concourse.tile as tile
from concourse.bass_utils import run_bass_kernel_spmd

B, S, D, H = 2, 2048, 1024, 16
DK = 64
NCORES = 8
HPC = 4           # heads per core
QC = HPC * DK     # 256 projected dims per core
P = 128
SQT = 512         # sq tile width
NQT = S // SQT    # 4
NKT = S // P      # 16 sk chunks
KC = D // P       # 8 contraction chunks for projections
VW = 65           # v columns per head incl. ones column
VWC = HPC * VW    # 260

F32 = mybir.dt.float32
F32R = mybir.dt.float32r
F16 = mybir.dt.float16
AF = mybir.ActivationFunctionType
ALU = mybir.AluOpType

POSR_BUFS = 2
SCORE_BUFS = 2


def build_program(repeat=1):
    nc = bacc.Bacc()
    xq = nc.dram_tensor("xq", [D, S], F16, kind="ExternalInput")
    xk = nc.dram_tensor("xk", [D, S], F16, kind="ExternalInput")
    xv = nc.dram_tensor("xv", [D, S], F16, kind="ExternalInput")
    posr = nc.dram_tensor("posr", [S, S], F16, kind="ExternalInput")  # exp(posr^T) fp16
    wq = nc.dram_tensor("wq", [D, QC], F16, kind="ExternalInput")
    wk = nc.dram_tensor("wk", [D, QC], F16, kind="ExternalInput")
    wv = nc.dram_tensor("wv", [D, VWC], F16, kind="ExternalInput")
    wo = nc.dram_tensor("wo", [QC, D], F32R, kind="ExternalInput")
    bq = nc.dram_tensor("bq", [2, P], F32, kind="ExternalInput")
    bk = nc.dram_tensor("bk", [2, P], F32, kind="ExternalInput")
    bv = nc.dram_tensor("bv", [1, VWC], F32, kind="ExternalInput")
    out = nc.dram_tensor("out", [S, D], F16, kind="ExternalOutput")

    with tile.TileContext(nc) as tc:
        with (
            tc.tile_pool(name="const", bufs=1) as cp,
            tc.tile_pool(name="qkv", bufs=1) as qp,
            tc.tile_pool(name="psum", bufs=1, space="PSUM") as pp,
        ):
            # ---- phase 1: weights ----
            wq_sb = cp.tile([P, KC, QC], F16)
            wk_sb = cp.tile([P, KC, QC], F16)
            wv_sb = cp.tile([P, KC, VWC], F16)
            wo_sb = cp.tile([P, QC // P, D], F32R)
            nc.sync.dma_start(wq_sb[:], wq.rearrange("(kc p) m -> p kc m", p=P))
            nc.sync.dma_start(wk_sb[:], wk.rearrange("(kc p) m -> p kc m", p=P))
            nc.sync.dma_start(wv_sb[:], wv.rearrange("(kc p) m -> p kc m", p=P))
            nc.sync.dma_start(wo_sb[:], wo.rearrange("(kc p) m -> p kc m", p=P))
            bq_sb = cp.tile([P, 2], F32)
            bk_sb = cp.tile([P, 2], F32)
            nc.sync.dma_start(bq_sb[:], bq.rearrange("t p -> p t"))
            nc.sync.dma_start(bk_sb[:], bk.rearrange("t p -> p t"))
            bv1 = cp.tile([1, VWC], F32)
            nc.sync.dma_start(bv1[:], bv[:])
            bv_sb = cp.tile([P, VWC], F32)
            nc.gpsimd.partition_broadcast(bv_sb[:], bv1[:])

            qT = qp.tile([P, 2, S], F32R)
            kT = qp.tile([P, 2, S], F32R)
            ctxT = qp.tile([P, 2, S], F32R)
            v_aug = qp.tile([P, NKT, VWC], F32R)

            for rep in range(repeat):
                rep_stack = contextlib.ExitStack()
                posp = rep_stack.enter_context(
                    tc.tile_pool(name=f"posr{rep}", bufs=2))
                # ---- phase 2: projections (k first so attention can start
                # as soon as q's first tile lands) ----
                with tc.tile_pool(name=f"xs{rep}", bufs=4) as xp:
                    for nt in range(NQT):
                        sq = slice(nt * SQT, (nt + 1) * SQT)
                        xq_t = xp.tile([P, KC, SQT], F16, tag="x",
                                       name=f"xq_{rep}_{nt}")
                        xk_t = xp.tile([P, KC, SQT], F16, tag="x",
                                       name=f"xk_{rep}_{nt}")
                        xv_t = xp.tile([P, KC, SQT], F16, tag="x",
                                       name=f"xv_{rep}_{nt}")
                        nc.sync.dma_start(
                            xk_t[:],
                            xk.rearrange("(kc p) s -> p kc s", p=P)[:, :, sq])
                        nc.sync.dma_start(
                            xq_t[:],
                            xq.rearrange("(kc p) s -> p kc s", p=P)[:, :, sq])
                        nc.sync.dma_start(
                            xv_t[:],
                            xv.rearrange("(kc p) s -> p kc s", p=P)[:, :, sq])
                        for mt in range(2):
                            ms = slice(mt * P, (mt + 1) * P)
                            ps_k = pp.tile([P, SQT], F32, tag="mm", bufs=2,
                                           name=f"ps_k_{rep}_{nt}_{mt}")
                            for kc in range(KC):
                                nc.tensor.matmul(ps_k[:], wk_sb[:, kc, ms],
                                                 xk_t[:, kc, :],
                                                 start=kc == 0,
                                                 stop=kc == KC - 1)
                            nc.vector.tensor_scalar_add(kT[:, mt, sq], ps_k[:],
                                                        bk_sb[:, mt:mt + 1])
                            ps_q = pp.tile([P, SQT], F32, tag="mm", bufs=2,
                                           name=f"ps_q_{rep}_{nt}_{mt}")
                            for kc in range(KC):
                                nc.tensor.matmul(ps_q[:], wq_sb[:, kc, ms],
                                                 xq_t[:, kc, :],
                                                 start=kc == 0,
                                                 stop=kc == KC - 1)
                            nc.vector.tensor_scalar_add(qT[:, mt, sq], ps_q[:],
                                                        bq_sb[:, mt:mt + 1])
                        for st in range(SQT // P):
                            sc = nt * (SQT // P) + st
                            ps_v = pp.tile([P, VWC], F32, tag="aux", bufs=2,
                                           name=f"ps_v_{rep}_{sc}")
                            for kc in range(KC):
                                nc.tensor.matmul(ps_v[:],
                                                 xv_t[:, kc,
                                                      st * P:(st + 1) * P],
                                                 wv_sb[:, kc, :],
                                                 start=kc == 0,
                                                 stop=kc == KC - 1)
                            nc.vector.tensor_tensor(
                                v_aug[:, sc, :], ps_v[:], bv_sb[:], ALU.add)

                # ---- phase 3+4: pipelined attention ----
                with contextlib.ExitStack() as wps:
                    wp = wps.enter_context(
                        tc.tile_pool(name=f"work{rep}", bufs=2))
                    posr_r = posr.rearrange("(kt p) q -> p kt q", p=P)
                    pos_tiles = {}

                    def emit_scores(qt, h):
                        sq = slice(qt * SQT, (qt + 1) * SQT)
                        if h == 0 and qt not in pos_tiles:
                            pos_tiles[qt] = []
                            for g in range(2):
                                pt = posp.tile([P, NKT // 2, SQT], F16,
                                               tag="posr", bufs=POSR_BUFS,
                                               name=f"pos_{rep}_{qt}_{g}")
                                nc.sync.dma_start(
                                    pt[:],
                                    posr_r[:,
                                           g * (NKT // 2):(g + 1) * (NKT // 2),
                                           sq])
                                pos_tiles[qt].append(pt)
                        hp = slice((h % 2) * DK, (h % 2) * DK + DK)
                        hm = h // 2
                        scores = wp.tile([P, NKT, SQT], F32R, tag="scores",
                                         bufs=SCORE_BUFS,
                                         name=f"sc_{rep}_{qt}_{h}")
                        for j in range(NKT // 2):
                            ps_s = pp.tile([P, 2, SQT], F32, tag="s2", bufs=2,
                                           name=f"ps_s_{rep}_{qt}_{h}_{j}")
                            for u in range(2):
                                kt = 2 * j + u
                                nc.tensor.matmul(
                                    ps_s[:, u, :],
                                    kT[hp, hm, kt * P:(kt + 1) * P],
                                    qT[hp, hm, sq], start=True, stop=True)
                            nc.scalar.activation(scores[:, 2 * j:2 * j + 2, :],
                                                 ps_s[:], AF.Exp)
                        for g in range(2):
                            for q4 in range(2):
                                qtr = slice((2 * g + q4) * (NKT // 4),
                                            (2 * g + q4 + 1) * (NKT // 4))
                                pq4 = slice(q4 * (NKT // 4),
                                            (q4 + 1) * (NKT // 4))
                                nc.vector.tensor_tensor(
                                    scores[:, qtr, :], scores[:, qtr, :],
                                    pos_tiles[qt][g][:, pq4, :], ALU.mult)
                        return scores

                    def emit_ctx(qt, h, scores):
                        sq = slice(qt * SQT, (qt + 1) * SQT)
                        hp = slice((h % 2) * DK, (h % 2) * DK + DK)
                        hm = h // 2
                        ps_c = pp.tile([VW, SQT], F32, tag="aux", bufs=2,
                                       name=f"ps_c_{rep}_{qt}_{h}")
                        for kt in range(NKT):
                            nc.tensor.matmul(
                                ps_c[:], v_aug[:, kt, h * VW:(h + 1) * VW],
                                scores[:, kt, :],
                                start=kt == 0, stop=kt == NKT - 1)
                        rec = wp.tile([1, SQT], F32, tag="rec", bufs=1,
                                      name=f"rec_{rep}_{qt}_{h}")
                        nc.vector.reciprocal(rec[:], ps_c[DK:VW, :])
                        bc = wp.tile([DK, SQT], F32, tag="bc", bufs=1,
                                     name=f"bc_{rep}_{qt}_{h}")
                        nc.gpsimd.partition_broadcast(bc[:], rec[:])
                        nc.vector.tensor_tensor(ctxT[hp, hm, sq],
                                                ps_c[:DK, :], bc[:], ALU.mult)

                    def emit_outproj(qt):
                        for mt4 in range(SQT // P):
                            mt = qt * (SQT // P) + mt4
                            ms = slice(mt * P, (mt + 1) * P)
                            ot = wp.tile([P, D], F16, tag="o", bufs=2,
                                         name=f"ot_{rep}_{mt}")
                            for nt2 in range(D // SQT):
                                ns = slice(nt2 * SQT, (nt2 + 1) * SQT)
                                ps_o = pp.tile([P, SQT], F32, tag="mm",
                                               bufs=2,
                                               name=f"ps_o_{rep}_{mt}_{nt2}")
                                for kc2 in range(QC // P):
                                    nc.tensor.matmul(
                                        ps_o[:], ctxT[:, kc2, ms],
                                        wo_sb[:, kc2, ns],
                                        start=kc2 == 0,
                                        stop=kc2 == QC // P - 1)
                                nc.vector.tensor_copy(ot[:, ns], ps_o[:])
                            nc.sync.dma_start(out[ms, :], ot[:])

                    units = [(qt, h) for qt in range(NQT) for h in range(HPC)]
                    pending = None  # (qt, h, scores)
                    for qt, h in units:
                        if pending is not None and h == 0:
                            # posr tiles for qt arrive before draining pending
                            pos_tiles[qt] = []
                            for g in range(2):
                                pt = posp.tile([P, NKT // 2, SQT], F16,
                                               tag="posr", bufs=POSR_BUFS,
                                               name=f"pos_{rep}_{qt}_{g}")
                                nc.sync.dma_start(
                                    pt[:],
                                    posr_r[:,
                                           g * (NKT // 2):(g + 1) * (NKT // 2),
                                           slice(qt * SQT, (qt + 1) * SQT)])
                                pos_tiles[qt].append(pt)
                        if pending is not None:
                            pq, ph, psc = pending
                            emit_ctx(pq, ph, psc)
                            if ph == HPC - 1:
                                emit_outproj(pq)
                        scores = emit_scores(qt, h)
                        pending = (qt, h, scores)
                    pq, ph, psc = pending
                    emit_ctx(pq, ph, psc)
                    emit_outproj(pq)
                rep_stack.close()

    nc.compile()
    return nc


def _augment_wv(Wv, qs):
    wv_c = np.zeros((D, VWC), dtype=np.float32)
    blk = Wv[qs].T  # [D, QC]
    for h in range(HPC):
        wv_c[:, h * VW:h * VW + DK] = blk[:, h * DK:(h + 1) * DK]
    return wv_c


def _augment_bv(bv, qs):
    bv_c = np.zeros((1, VWC), dtype=np.float32)
    blk = np.asarray(bv[qs], dtype=np.float32)
    for h in range(HPC):
        bv_c[0, h * VW:h * VW + DK] = blk[h * DK:(h + 1) * DK]
        bv_c[0, h * VW + DK] = 1.0
    return bv_c


def make_in_maps(query, key, value, posr, Wq, bq, Wk, bk, Wv, bv, Wo):
    scale = 1.0 / np.sqrt(DK)
    in_maps = []
    for b in range(B):
        xq = np.ascontiguousarray(query[b].T, dtype=np.float16)
        xk = np.ascontiguousarray(key[b].T, dtype=np.float16)
        xv = np.ascontiguousarray(value[b].T, dtype=np.float16)
        pr = np.exp(np.asarray(posr[b].T, dtype=np.float32)).astype(np.float16)
        for hg in range(4):
            qs = slice(hg * QC, (hg + 1) * QC)
            in_maps.append({
                "xq": xq, "xk": xk, "xv": xv, "posr": pr,
                "wq": np.ascontiguousarray(Wq[qs].T * scale, dtype=np.float16),
                "wk": np.ascontiguousarray(Wk[qs].T, dtype=np.float16),
                "wv": _augment_wv(Wv, qs).astype(np.float16),
                "wo": np.ascontiguousarray(Wo[:, qs].T, dtype=np.float32),
                "bq": (np.asarray(bq[qs], dtype=np.float32) * scale
                       ).reshape(2, P),
                "bk": np.asarray(bk[qs], dtype=np.float32).reshape(2, P),
                "bv": _augment_bv(bv, qs),
            })
    return in_maps


_nc_cache = []


def get_program():
    if not _nc_cache:
        _nc_cache.append(build_program())
    return _nc_cache[0]


def kernel(query, key, value, mask, posr, Wq, bq, Wk, bk, Wv, bv, Wo, bo):
    query = np.asarray(query)
    nc = get_program()
    in_maps = make_in_maps(np.asarray(query), np.asarray(key),
                           np.asarray(value), np.asarray(posr),
                           np.asarray(Wq), np.asarray(bq), np.asarray(Wk),
                           np.asarray(bk), np.asarray(Wv), np.asarray(bv),
                           np.asarray(Wo))
    res = run_bass_kernel_spmd(nc, in_maps, core_ids=list(range(NCORES)))
    bo = np.asarray(bo, dtype=np.float32)
    outs = []
    for b in range(B):
        acc = res.results[4 * b]["out"].astype(np.float32).copy()
        for hg in range(1, 4):
            acc += res.results[4 * b + hg]["out"]
        outs.append(acc + bo[None, :])
    return np.stack(outs).astype(np.float32)



# revision 47
# speedup vs baseline: 94.6751x; 94.6751x over previous
"""Multi-headed attention with additive positional bias on 8 Trainium2 cores.

Sharding: data-parallel over batch (B=2) x tensor-parallel over head groups
(4 heads per core).  core = b*4 + hg handles batch b, heads [4*hg, 4*hg+4).

Per-core kernel (all matmul operands fp16; PSUM accumulation fp32):
  phase 1: load weight shards (pre-transposed on host) into SBUF
  phase 2: projections, K first then V then Q so attention can start as
      early as possible.
      qT/kT in head-transposed layout [qdim(128part) x 2 x S] fp16
      v packed as v_aug [sk(128part) x 16 x (4 heads x 65)] fp16 with a
      ones-column per head (softmax denominator trick via an extra zero
      weight column + bias=1 in the V projection)
  phase 3 (software-pipelined across units u=(sq-tile, head-pair)):
      per kt: two K=64 score matmuls (even head -> PE row-tile (0,0),
      odd head -> (64,0)) write the two banks of one PSUM tile and run
      concurrently on the two halves of the PE array; ACT exps the pair
      [128,2,512] PSUM -> fp16 scores; DVE multiplies by exp(posr^T)
      (fp16 2x mode).  ctx matmuls + normalization for unit u are
      emitted one unit later so the PE runs unit u+1's score matmuls
      while ACT/DVE process unit u.
      ctxT_aug = v_aug_h^T p (accumulate over sk); normalize by the
      ones-row denominator (reciprocal_approx_fast + gpsimd partition
      broadcast).
  phase 4: out = ctxT^T @ WoT shard per sq-tile as soon as its 4 heads
      finish (gpsimd evacuates PSUM), DMA to DRAM.

Host: fold 1/sqrt(dk) into Wq/bq, transpose inputs, sum the 4 per-batch
partial outputs + bo.  mask is all-ones by construction and ignored.
"""
import contextlib

import numpy as np

import concourse.bacc as bacc
import concourse.mybir as mybir
import concourse.tile as tile
from concourse.bass_utils import run_bass_kernel_spmd

B, S, D, H = 2, 2048, 1024, 16
DK = 64
NCORES = 8
HPC = 4           # heads per core
QC = HPC * DK     # 256 projected dims per core
P = 128
SQT = 512         # sq tile width
NQT = S // SQT    # 4
NKT = S // P      # 16 sk chunks
KC = D // P       # 8 contraction chunks for projections
VW = 65           # v columns per head incl. ones column
VWC = HPC * VW    # 260

F32 = mybir.dt.float32
F16 = mybir.dt.float16
AF = mybir.ActivationFunctionType
ALU = mybir.AluOpType

POSR_BUFS = 2
SCORE_BUFS = 2


def build_program(repeat=1):
    # All inputs arrive pre-tiled (partition-major) so every DMA is a
    # contiguous-per-partition 2D pattern: ~128 descriptors, sub-us issue.
    nc = bacc.Bacc()
    xq = nc.dram_tensor("xq", [NQT, P, KC, SQT], F16, kind="ExternalInput")
    xk = nc.dram_tensor("xk", [NQT, P, KC, SQT], F16, kind="ExternalInput")
    xv = nc.dram_tensor("xv", [NQT, P, KC, SQT], F16, kind="ExternalInput")
    # exp(posr^T - 3) fp16, laid out [qt, p, kt, sq]
    posr = nc.dram_tensor("posr", [NQT, P, NKT, SQT], F16,
                          kind="ExternalInput")
    wq = nc.dram_tensor("wq", [P, KC, QC], F16, kind="ExternalInput")
    wk = nc.dram_tensor("wk", [P, KC, QC], F16, kind="ExternalInput")
    wv = nc.dram_tensor("wv", [P, KC, VWC], F16, kind="ExternalInput")
    wo = nc.dram_tensor("wo", [P, QC // P, D], F16, kind="ExternalInput")
    bq = nc.dram_tensor("bq", [P, 2], F32, kind="ExternalInput")
    bk = nc.dram_tensor("bk", [P, 2], F32, kind="ExternalInput")
    bv = nc.dram_tensor("bv", [1, VWC], F32, kind="ExternalInput")
    out = nc.dram_tensor("out", [S, D], F16, kind="ExternalOutput")

    with tile.TileContext(nc) as tc:
        with (
            tc.tile_pool(name="const", bufs=1) as cp,
            tc.tile_pool(name="qkv", bufs=1) as qp,
            tc.tile_pool(name="psum", bufs=1, space="PSUM") as pp,
        ):
            # ---- phase 1: weights ----
            wq_sb = cp.tile([P, KC, QC], F16)
            wk_sb = cp.tile([P, KC, QC], F16)
            wv_sb = cp.tile([P, KC, VWC], F16)
            wo_sb = cp.tile([P, QC // P, D], F16)
            bq_sb = cp.tile([P, 2], F32)
            bk_sb = cp.tile([P, 2], F32)
            bv1 = cp.tile([1, VWC], F32)
            # weights issue on the ACT-engine DGE so the sync engine can
            # issue xk immediately; first matmul waits only wk+xk
            nc.scalar.dma_start(wk_sb[:], wk[:])
            nc.sync.dma_start(bk_sb[:], bk[:])
            nc.sync.dma_start(bq_sb[:], bq[:])
            nc.sync.dma_start(bv1[:], bv[:])
            nc.scalar.dma_start(wv_sb[:], wv[:])
            nc.scalar.dma_start(wq_sb[:], wq[:])
            nc.scalar.dma_start(wo_sb[:], wo[:])
            bv_sb = cp.tile([P, VWC], F32)
            nc.gpsimd.partition_broadcast(bv_sb[:], bv1[:])

            qT = qp.tile([P, 2, S], F16)
            kT = qp.tile([P, 2, S], F16)
            ctxT = qp.tile([P, 2, S], F16)
            v_aug = qp.tile([P, NKT, VWC], F16)

            for rep in range(repeat):
                rep_stack = contextlib.ExitStack()
                posp = rep_stack.enter_context(
                    tc.tile_pool(name=f"posr{rep}", bufs=2))
                xp = rep_stack.enter_context(
                    tc.tile_pool(name=f"xs{rep}", bufs=2))

                # ---- phase 2: projections ----
                # x streams through 2 rotating 8KB tiles per input; only
                # the K projections run up front — V and Q projections are
                # PE filler blocks inside the first attention unit.
                def x_tile(which, dram, nt, engine):
                    t = xp.tile([P, KC, SQT], F16, tag="x",
                                name=f"x{which}_{rep}_{nt}")
                    engine.dma_start(t[:], dram[nt])
                    return t

                # xk on the sync DGE; xq0 + xv + xq1-3 on the ACT DGE so
                # the two input streams load in parallel
                xk_t = [x_tile("k", xk, nt, nc.sync) for nt in range(NQT)]
                xq_t = {0: x_tile("q", xq, 0, nc.scalar)}
                xv_t = {nt: x_tile("v", xv, nt, nc.scalar)
                        for nt in range(NQT)}
                for nt in range(1, NQT):
                    xq_t[nt] = x_tile("q", xq, nt, nc.scalar)

                def emit_kproj(nt):
                    sq = slice(nt * SQT, (nt + 1) * SQT)
                    for mt in range(2):
                        ms = slice(mt * P, (mt + 1) * P)
                        ps_k = pp.tile([P, SQT], F32, tag="mm", bufs=2,
                                       name=f"ps_k_{rep}_{nt}_{mt}")
                        for kc in range(KC):
                            nc.tensor.matmul(ps_k[:], wk_sb[:, kc, ms],
                                             xk_t[nt][:, kc, :],
                                             start=kc == 0,
                                             stop=kc == KC - 1)
                        nc.vector.tensor_scalar_add(kT[:, mt, sq], ps_k[:],
                                                    bk_sb[:, mt:mt + 1])

                def emit_vproj(nt):
                    for st in range(SQT // P):
                        sc = nt * (SQT // P) + st
                        ps_v = pp.tile([P, VWC], F32, tag="mm", bufs=2,
                                       name=f"ps_v_{rep}_{sc}")
                        for kc in range(KC):
                            nc.tensor.matmul(ps_v[:],
                                             xv_t[nt][:, kc,
                                                      st * P:(st + 1) * P],
                                             wv_sb[:, kc, :],
                                             start=kc == 0,
                                             stop=kc == KC - 1)
                        nc.vector.tensor_tensor(
                            v_aug[:, sc, :], ps_v[:], bv_sb[:], ALU.add)

                def emit_qproj(nt):
                    sq = slice(nt * SQT, (nt + 1) * SQT)
                    for mt in range(2):
                        ms = slice(mt * P, (mt + 1) * P)
                        ps_q = pp.tile([P, SQT], F32, tag="mm", bufs=2,
                                       name=f"ps_q_{rep}_{nt}_{mt}")
                        for kc in range(KC):
                            nc.tensor.matmul(ps_q[:], wq_sb[:, kc, ms],
                                             xq_t[nt][:, kc, :],
                                             start=kc == 0,
                                             stop=kc == KC - 1)
                        nc.vector.tensor_scalar_add(qT[:, mt, sq], ps_q[:],
                                                    bq_sb[:, mt:mt + 1])

                for nt in range(NQT):
                    emit_kproj(nt)
                emit_qproj(0)

                # ---- phase 3+4: pipelined attention over (qt, head-pair) ----
                with contextlib.ExitStack() as wps:
                    wp = wps.enter_context(
                        tc.tile_pool(name=f"work{rep}", bufs=2))
                    pos_tiles = {}

                    def emit_pos_dma(qt):
                        # qt0 posr rides the gpsimd SWDGE: the sync queue
                        # streams xk and the ACT DGE streams xv/xq at start
                        eng = nc.gpsimd if qt == 0 else nc.sync
                        pos_tiles[qt] = []
                        for g in range(2):
                            pt = posp.tile([P, NKT // 2, SQT], F16,
                                           tag="posr", bufs=POSR_BUFS,
                                           name=f"pos_{rep}_{qt}_{g}")
                            eng.dma_start(
                                pt[:],
                                posr[qt, :,
                                     g * (NKT // 2):(g + 1) * (NKT // 2), :])
                            pos_tiles[qt].append(pt)

                    # External PE filler blocks (out-proj of the previous
                    # qt, deferred q projections) are popped between score
                    # groups; ctx matmuls self-trail inside their own unit.
                    filler_q = []

                    def emit_unit(qt, hm, self_trail=True):
                        # head pair (2*hm, 2*hm+1): even head on PE rows
                        # 0-63, odd head on rows 64-127, concurrent tiles.
                        sq = slice(qt * SQT, (qt + 1) * SQT)
                        if qt not in pos_tiles:
                            emit_pos_dma(qt)
                        scores = wp.tile([P, NKT, 2, SQT], F16, tag="scores",
                                         bufs=SCORE_BUFS,
                                         name=f"sc_{rep}_{qt}_{hm}")
                        ps_cs = {}

                        def ctx_pair(t):
                            # ctx matmuls for kt pair t, both heads
                            for e in range(2):
                                h = 2 * hm + e
                                if t == 0:
                                    ps_cs[e] = pp.tile(
                                        [VW, SQT], F32, tag="aux", bufs=2,
                                        name=f"ps_c_{rep}_{qt}_{h}")
                                for kt in (2 * t, 2 * t + 1):
                                    nc.tensor.matmul(
                                        ps_cs[e][:],
                                        v_aug[:, kt, h * VW:(h + 1) * VW],
                                        scores[:, kt, e, :],
                                        start=kt == 0,
                                        stop=kt == NKT - 1)

                        def emit_norm():
                            for e in range(2):
                                h = 2 * hm + e
                                hp = slice(e * DK, (e + 1) * DK)
                                ps_c = ps_cs[e]
                                den = wp.tile([1, SQT], F32, tag="den",
                                              bufs=1,
                                              name=f"den_{rep}_{qt}_{h}")
                                nc.vector.tensor_copy(den[:], ps_c[DK:VW, :])
                                rec = wp.tile([1, SQT], F32, tag="rec",
                                              bufs=1,
                                              name=f"rec_{rep}_{qt}_{h}")
                                nc.vector.reciprocal_approx_fast(
                                    rec[:], den[:])
                                bc = wp.tile([DK, SQT], F32, tag="bc",
                                             bufs=1,
                                             name=f"bc_{rep}_{qt}_{h}")
                                nc.gpsimd.partition_broadcast(bc[:], rec[:])
                                nc.vector.tensor_tensor(ctxT[hp, hm, sq],
                                                        ps_c[:DK, :], bc[:],
                                                        ALU.mult)

                        for kt in range(NKT):
                            ks = slice(kt * P, (kt + 1) * P)
                            ps_s = pp.tile([P, 2, SQT], F32, tag="s2",
                                           bufs=2,
                                           name=f"ps_{rep}_{qt}_{hm}_{kt}")
                            nc.tensor.matmul(ps_s[:, 0, :],
                                             kT[0:DK, hm, ks],
                                             qT[0:DK, hm, sq],
                                             start=True, stop=True)
                            nc.tensor.matmul(ps_s[:, 1, :],
                                             kT[DK:P, hm, ks],
                                             qT[DK:P, hm, sq],
                                             start=True, stop=True)
                            nc.scalar.activation(
                                scores[:, kt, :, :], ps_s[:], AF.Exp)
                            if kt % 4 == 3:
                                g = kt // 4
                                qtr = slice(4 * g, 4 * g + 4)
                                pq4 = slice((g % 2) * 4, (g % 2) * 4 + 4)
                                for e in range(2):
                                    nc.vector.tensor_tensor(
                                        scores[:, qtr, e, :],
                                        scores[:, qtr, e, :],
                                        pos_tiles[qt][g // 2][:, pq4, :],
                                        ALU.mult)
                                for _ in range(2):
                                    if filler_q:
                                        filler_q.pop(0)()
                                if self_trail and g >= 1:
                                    ctx_pair(2 * g - 2)
                                    ctx_pair(2 * g - 1)
                        if self_trail:
                            # defer the unit's tail (last ctx pairs +
                            # normalize) into the filler queue: it pops
                            # early in the NEXT unit, between score groups,
                            # instead of clumping at the unit boundary
                            # where it would starve the ACT exp stream
                            def tail_blk():
                                ctx_pair(6)
                                ctx_pair(7)
                                emit_norm()
                            filler_q.append(tail_blk)
                        else:
                            # first unit: v_aug is still being produced by
                            # the v-projection fillers, so its ctx trails
                            # as external fillers instead
                            for t0 in range(0, 8, 2):
                                def blk(t0=t0):
                                    ctx_pair(t0)
                                    ctx_pair(t0 + 1)
                                    if t0 == 6:
                                        emit_norm()
                                filler_q.append(blk)

                    def make_outproj_fillers(qt):
                        def block(mt4):
                            def emit():
                                mt = qt * (SQT // P) + mt4
                                ms = slice(mt * P, (mt + 1) * P)
                                ot = wp.tile([P, D], F16, tag="o", bufs=2,
                                             name=f"ot_{rep}_{mt}")
                                for nt2 in range(D // SQT):
                                    ns = slice(nt2 * SQT, (nt2 + 1) * SQT)
                                    ps_o = pp.tile(
                                        [P, SQT], F32, tag="mm", bufs=2,
                                        name=f"ps_o_{rep}_{mt}_{nt2}")
                                    for kc2 in range(QC // P):
                                        nc.tensor.matmul(
                                            ps_o[:], ctxT[:, kc2, ms],
                                            wo_sb[:, kc2, ns],
                                            start=kc2 == 0,
                                            stop=kc2 == QC // P - 1)
                                    nc.vector.tensor_copy(ot[:, ns], ps_o[:])
                                nc.sync.dma_start(out[ms, :], ot[:])
                            return emit

                        return [block(mt4) for mt4 in range(SQT // P)]

                    # deferred V/Q projections run as PE fillers inside the
                    # first attention units (their inputs stream in behind
                    # the K projections)
                    for nt in range(NQT):
                        filler_q.append(
                            (lambda n: lambda: emit_vproj(n))(nt))
                    for nt in range(1, NQT):
                        filler_q.append(
                            (lambda n: lambda: emit_qproj(n))(nt))

                    units = [(qt, hm) for qt in range(NQT) for hm in range(2)]
                    for qt, hm in units:
                        if hm == 0:
                            if qt not in pos_tiles:
                                emit_pos_dma(qt)
                            if qt + 1 < NQT:
                                # prefetch next qt's posr during this unit
                                emit_pos_dma(qt + 1)
                        # qt0 units' ctx runs as external fillers: v_aug is
                        # still streaming in, and the aux-psum rotation
                        # requires a unit's ctx to fully drain before the
                        # next self-trailing unit allocates its banks
                        emit_unit(qt, hm, self_trail=qt >= 1)
                        if hm == 1:
                            filler_q.extend(make_outproj_fillers(qt))
                    while filler_q:
                        filler_q.pop(0)()
                rep_stack.close()

    nc.compile()
    return nc


def _augment_wv(Wv, qs):
    wv_c = np.zeros((D, VWC), dtype=np.float32)
    blk = Wv[qs].T  # [D, QC]
    for h in range(HPC):
        wv_c[:, h * VW:h * VW + DK] = blk[:, h * DK:(h + 1) * DK]
    return wv_c


def _augment_bv(bv, qs):
    bv_c = np.zeros((1, VWC), dtype=np.float32)
    blk = np.asarray(bv[qs], dtype=np.float32)
    for h in range(HPC):
        bv_c[0, h * VW:h * VW + DK] = blk[h * DK:(h + 1) * DK]
        bv_c[0, h * VW + DK] = 1.0
    return bv_c


def _ptile(arr_ds):
    # [D, n] -> [P, KC, n] partition-major tiling (d = kc*P + p)
    n = arr_ds.shape[1]
    return np.ascontiguousarray(
        arr_ds.reshape(KC, P, n).transpose(1, 0, 2))


def _ptile4(arr_ds):
    # [D, S] -> [NQT, P, KC, SQT] per-nt partition-major tiling
    return np.ascontiguousarray(
        arr_ds.reshape(KC, P, NQT, SQT).transpose(2, 1, 0, 3))


def make_in_maps(query, key, value, posr, Wq, bq, Wk, bk, Wv, bv, Wo):
    scale = 1.0 / np.sqrt(DK)
    in_maps = []
    for b in range(B):
        xq = _ptile4(np.asarray(query[b].T, dtype=np.float16))
        xk = _ptile4(np.asarray(key[b].T, dtype=np.float16))
        xv = _ptile4(np.asarray(value[b].T, dtype=np.float16))
        # -3 shift keeps exp(qk)*exp(posr) within fp16 range (max combined
        # score ~11.2 > ln 65504); the constant cancels in the softmax.
        pr = np.exp(np.asarray(posr[b].T, dtype=np.float32) - 3.0
                    ).astype(np.float16)
        # [S(k), S(q)] -> [qt, p, kt, sq]
        pr = np.ascontiguousarray(
            pr.reshape(NKT, P, NQT, SQT).transpose(2, 1, 0, 3))
        for hg in range(4):
            qs = slice(hg * QC, (hg + 1) * QC)
            wq_c = np.asarray(Wq[qs].T * scale, dtype=np.float16)
            wk_c = np.asarray(Wk[qs].T, dtype=np.float16)
            wo_c = np.asarray(Wo[:, qs].T, dtype=np.float16)
            in_maps.append({
                "xq": xq, "xk": xk, "xv": xv, "posr": pr,
                "wq": _ptile(wq_c),
                "wk": _ptile(wk_c),
                "wv": _ptile(_augment_wv(Wv, qs).astype(np.float16)),
                "wo": np.ascontiguousarray(
                    wo_c.reshape(QC // P, P, D).transpose(1, 0, 2)),
                "bq": (np.asarray(bq[qs], dtype=np.float32) * scale
                       ).reshape(2, P).T.copy(),
                "bk": np.asarray(bk[qs], dtype=np.float32
                                 ).reshape(2, P).T.copy(),
                "bv": _augment_bv(bv, qs),
            })
    return in_maps


_nc_cache = []


def get_program():
    if not _nc_cache:
        _nc_cache.append(build_program())
    return _nc_cache[0]


def kernel(query, key, value, mask, posr, Wq, bq, Wk, bk, Wv, bv, Wo, bo):
    query = np.asarray(query)
    nc = get_program()
    in_maps = make_in_maps(np.asarray(query), np.asarray(key),
                           np.asarray(value), np.asarray(posr),
                           np.asarray(Wq), np.asarray(bq), np.asarray(Wk),
                           np.asarray(bk), np.asarray(Wv), np.asarray(bv),
                           np.asarray(Wo))
    res = run_bass_kernel_spmd(nc, in_maps, core_ids=list(range(NCORES)))
    bo = np.asarray(bo, dtype=np.float32)
    outs = []
    for b in range(B):
        acc = res.results[4 * b]["out"].astype(np.float32).copy()
        for hg in range(1, 4):
            acc += res.results[4 * b + hg]["out"]
        outs.append(acc + bo[None, :])
    return np.stack(outs).astype(np.float32)
